# revision 12
# baseline (speedup 1.0000x reference)
"""Single-head causal attention with RoPE on 8 Trainium2 NeuronCores.

Problem: x:(8,2048,1024), Wq/Wk/Wv:(1024,64) -> out:(8,2048,64)
  q = rope(x@Wq); k = rope(x@Wk); v = x@Wv
  out = softmax(causal(q k^T / sqrt(64))) @ v

Sharding: data-parallel over batch B=8, one batch element per core.

Per-core dataflow (transposed [feature, time] layout so the softmax
reduction lands on the PSUM partition dim via a ones-column matmul):
  A(i): qkT = [Wq|Wk]^T @ xT_chunk (PE, M=128); rot = R2 @ bf16(qkT) (PE)
        q'k' = qkT*coscos + rot*sinsin (DVE) -> qkrope bf16 (q rows 0:64,
        k rows 64:128); krope0 partition copy via SBUF-SBUF DMA
        v-proj for chunk PAIRS col-tiled: two chunks run CONCURRENTLY in
        PE column groups 0:64 / 64:128 (independent moving xbus streams)
  B(g): v natural via PE transpose -> vnat[128,16,65] (ones col 64)
  C(i): full s-blocks in PAIRS sharing one 2-bank PSUM tile; ONE exp
        ACTIVATE covers both (halves ACT per-call overhead); diagonal
        blocks single + causal-trimmed + trimask
        [num;den] psum += vnat_sb^T @ pexp (PE, M=65; ones col -> den)
        normalize: evict (DVE), PE-transpose, reciprocal + tensor_scalar
  DMA queues by need-time: xT stream on sync; consts on gpsimd (SWDGE,
  ordered by first use); krope0 dups on scalar; outputs on vector.
"""

import numpy as np
import ml_dtypes

B, T, C, H = 8, 2048, 1024, 64
NCORES = 8
CHUNK = 512
NCHUNK = T // CHUNK  # 4
NSB = T // 128       # 16 s-blocks
NCB = C // 128       # 8 c-blocks

bf16 = ml_dtypes.bfloat16


# ---------------------------------------------------------------- host consts
def _build_consts():
    half = H // 2
    inv_freq = (1.0 / (10000.0 ** (np.arange(half, dtype=np.float32) / half))).astype(
        np.float32
    )
    t = np.arange(T, dtype=np.float32)
    freqs = t[:, None] * inv_freq[None, :]  # (T, half) fp32
    cos = np.repeat(np.cos(freqs), 2, axis=-1)  # (T, H)
    sin = np.repeat(np.sin(freqs), 2, axis=-1)
    cosT = np.ascontiguousarray(cos.T)  # (H, T)
    sinT = np.ascontiguousarray(sin.T)

    coscos = np.concatenate([cosT, cosT], axis=0).astype(bf16)  # (128, T)
    sinsin = np.concatenate([sinT, sinT], axis=0).astype(bf16)

    # rot = R @ q with rot[2i] = -q[2i+1], rot[2i+1] = q[2i]
    Rm = np.zeros((H, H), np.float32)
    for i in range(half):
        Rm[2 * i, 2 * i + 1] = -1.0
        Rm[2 * i + 1, 2 * i] = 1.0
    r2 = np.zeros((128, 128), np.float32)
    r2[0:H, 0:H] = Rm.T
    r2[H:128, H:128] = Rm.T
    r2 = r2.astype(bf16)

    sl = np.arange(128)
    trimask = (sl[:, None] <= sl[None, :]).astype(bf16)  # (128, 128)

    identb2 = np.concatenate([np.eye(H), np.eye(H)], axis=0).astype(bf16)  # (128, 64)
    ident_f32 = np.eye(H + 1, dtype=np.float32)  # (65, 65) for out transposes

    return coscos, sinsin, r2, trimask, identb2, ident_f32


# ---------------------------------------------------------------- bass program
def _build_bass():
    import concourse.mybir as mybir
    import concourse.tile as tile
    from concourse import bacc
    from concourse.bass import ts

    BF = mybir.dt.bfloat16
    F32 = mybir.dt.float32
    Exp = mybir.ActivationFunctionType.Exp

    nc = bacc.Bacc(
        "TRN2",
        target_bir_lowering=False,
        debug=False,
        enable_asserts=False,
        num_devices=NCORES,
    )

    # xT prepacked on host to SBUF layout [128(p), chunk, cblk, 512]
    xT_d = nc.dram_tensor("xTp", [128, NCHUNK, NCB, CHUNK], BF, kind="ExternalInput")
    wqk_d = nc.dram_tensor("wqkp", [128, NCB, 128], BF, kind="ExternalInput")
    wv_d = nc.dram_tensor("wvp", [128, NCB, H], BF, kind="ExternalInput")
    r2_d = nc.dram_tensor("r2", [128, 128], BF, kind="ExternalInput")
    coscos_d = nc.dram_tensor("coscos", [128, T], BF, kind="ExternalInput")
    sinsin_d = nc.dram_tensor("sinsin", [128, T], BF, kind="ExternalInput")
    trimask_d = nc.dram_tensor("trimask", [128, 128], BF, kind="ExternalInput")
    identb_d = nc.dram_tensor("identb2", [128, H], BF, kind="ExternalInput")
    identf_d = nc.dram_tensor("identf", [H + 1, H + 1], F32, kind="ExternalInput")
    out_d = nc.dram_tensor("out", [T, H], F32, kind="ExternalOutput")

    with tile.TileContext(nc) as tc:
        with (
            tc.tile_pool(name="persist", bufs=1) as persist,
            tc.tile_pool(name="work", bufs=2) as work,
            tc.tile_pool(name="pexpp", bufs=4) as pexpp,
            tc.tile_pool(name="ps_scratch", bufs=2, space="PSUM") as ps_scratch,
            tc.tile_pool(name="ps_sc", bufs=2, space="PSUM") as ps_sc,
            tc.tile_pool(name="ps_out", bufs=2, space="PSUM") as ps_out,
        ):
            # ---- persistent SBUF tensors
            wqk_sb = persist.tile([128, NCB, 128], BF)
            wv_sb = persist.tile([128, NCB, H], BF)
            r2_sb = persist.tile([128, 128], BF)
            coscos_sb = persist.tile([128, T], BF)
            sinsin_sb = persist.tile([128, T], BF)
            trimask_sb = persist.tile([128, 128], BF)
            identb_sb = persist.tile([128, H], BF)
            identf_sb = persist.tile([H + 1, H + 1], F32)
            xT_sb = persist.tile([128, NCHUNK, NCB, CHUNK], BF)
            qkrope = persist.tile([128, T], BF)   # q' rows 0:64, k' rows 64:128
            krope0 = persist.tile([H, T], BF)     # k' copy at partitions 0:64
            vT_sb = persist.tile([128, NCHUNK // 2, CHUNK], BF)  # stacked pairs
            vnat = persist.tile([128, NSB, H + 1], BF)

            # ---- sync HWDGE queue: wqk (gates first matmul), then xT stream
            # (chunk 0 split in half so projections start ~1.3us earlier)
            nc.sync.dma_start(out=wqk_sb[:], in_=wqk_d.ap())
            nc.sync.dma_start(out=xT_sb[:, 0, 0:4], in_=xT_d.ap()[:, 0, 0:4])
            nc.sync.dma_start(out=xT_sb[:, 0, 4:8], in_=xT_d.ap()[:, 0, 4:8])
            for i in range(1, NCHUNK):
                nc.sync.dma_start(out=xT_sb[:, i], in_=xT_d.ap()[:, i])
            # ---- scalar HWDGE queue: rope consts + early weights
            nc.scalar.dma_start(out=coscos_sb[:], in_=coscos_d.ap())
            nc.scalar.dma_start(out=sinsin_sb[:], in_=sinsin_d.ap())
            nc.scalar.dma_start(out=r2_sb[:], in_=r2_d.ap())
            nc.scalar.dma_start(out=wv_sb[:], in_=wv_d.ap())
            # ---- gpsimd (SWDGE) queue: small late-need consts
            nc.gpsimd.dma_start(out=identb_sb[:], in_=identb_d.ap())
            nc.gpsimd.dma_start(out=trimask_sb[:], in_=trimask_d.ap())
            nc.gpsimd.dma_start(out=identf_sb[:], in_=identf_d.ap())

            nc.vector.memset(vnat[:], 1.0)  # ones col (64); cols 0:64 overwritten

            # PE warmup: ~3.5us of junk matmuls on a zeroed tile so the HAM
            # clock-gate opens to 2.4GHz before the first real projection
            zwarm = persist.tile([128, CHUNK], BF)
            nc.vector.memset(zwarm[:], 0.0)
            warm_ps = ps_sc.tile([128, 2, CHUNK], F32, tag="sc", name="warm")
            for w in range(16):
                nc.tensor.matmul(
                    warm_ps[:, 0, :],
                    zwarm[:, 0:128],
                    zwarm[:],
                    start=(w == 0),
                    stop=(w == 15),
                )

            def phase_a(i):
                tsl = ts(i, CHUNK)
                qk_ps = ps_scratch.tile([128, CHUNK], F32, tag="scr", name=f"qk{i}")
                for c in range(NCB):
                    nc.tensor.matmul(
                        qk_ps[:],
                        wqk_sb[:, c, :],
                        xT_sb[:, i, c, :],
                        start=(c == 0),
                        stop=(c == NCB - 1),
                    )
                qkS = work.tile([128, CHUNK], BF, tag="qkS", name=f"qkS{i}")
                nc.vector.tensor_copy(out=qkS[:], in_=qk_ps[:])
                rot_ps = ps_scratch.tile([128, CHUNK], F32, tag="scr", name=f"rot{i}")
                nc.tensor.matmul(rot_ps[:], r2_sb[:], qkS[:], start=True, stop=True)

                tmp1 = work.tile([128, CHUNK], BF, tag="tmp1", name=f"t1_{i}")
                nc.vector.tensor_mul(tmp1[:], qkS[:], coscos_sb[:, tsl])
                tmp2 = work.tile([128, CHUNK], BF, tag="tmp2", name=f"t2_{i}")
                nc.vector.tensor_mul(tmp2[:], rot_ps[:], sinsin_sb[:, tsl])
                nc.vector.tensor_add(qkrope[:, tsl], tmp1[:], tmp2[:])
                # k' partition copy for use as scores lhsT (gpsimd queue --
                # keeps the ACT instruction stream free for pure exp work)
                nc.gpsimd.dma_start(out=krope0[:, tsl], in_=qkrope[H:128, tsl])

                if i % 2 == 1:
                    # v-projection for chunk pair (i-1, i), col-tiled pairs
                    # interleaved so both column groups stream concurrently
                    g = i // 2
                    v_ps = ps_scratch.tile([128, CHUNK], F32, tag="scr", name=f"v{g}")
                    for c in range(NCB):
                        nc.tensor.matmul(
                            v_ps[0:H, :],
                            wv_sb[:, c, :],
                            xT_sb[:, i - 1, c, :],
                            start=(c == 0),
                            stop=(c == NCB - 1),
                            skip_group_check=True,
                        )
                        nc.tensor.matmul(
                            v_ps[H:128, :],
                            wv_sb[:, c, :],
                            xT_sb[:, i, c, :],
                            start=(c == 0),
                            stop=(c == NCB - 1),
                            skip_group_check=True,
                        )
                    nc.vector.tensor_copy(out=vT_sb[:, g, :], in_=v_ps[:])

            def phase_b(g):
                # transpose 8 s-blocks of the vT pair group g (chunks 2g, 2g+1)
                for half_ in range(2):
                    vn_ps = ps_out.tile(
                        [128, 4, H], BF, tag="out", name=f"vn{g}_{half_}"
                    )
                    base = H * half_
                    for j in range(4):
                        nc.tensor.transpose(
                            vn_ps[:, j, :],
                            vT_sb[base : base + H, g, ts(j, 128)],
                            identb_sb[base : base + H, :],
                        )
                    first = 8 * g + 4 * half_
                    nc.vector.tensor_copy(
                        out=vnat[:, first : first + 4, 0:H], in_=vn_ps[:]
                    )

            out_tiles = {}

            def phase_c_accum(i):
                nsb = 4 * i + 4  # s-blocks 0 .. 4i+3 (0..4i-1 full, then diag)
                out_ps = ps_out.tile([H + 1, CHUNK], F32, tag="out", name=f"o{i}")
                # full blocks in fused pairs
                for p in range(2 * i):
                    sc2 = ps_sc.tile([128, 2, CHUNK], F32, tag="sc", name=f"s{i}_{p}")
                    for h_ in range(2):
                        sb = 2 * p + h_
                        nc.tensor.matmul(
                            sc2[:, h_, :],
                            krope0[:, ts(sb, 128)],
                            qkrope[0:H, ts(i, CHUNK)],
                            start=True,
                            stop=True,
                        )
                    pexp2 = pexpp.tile(
                        [128, 2, CHUNK], BF, tag="pexp", name=f"p{i}_{p}"
                    )
                    nc.scalar.activation(
                        out=pexp2[:], in_=sc2[:], func=Exp, scale=0.125
                    )
                    for h_ in range(2):
                        sb = 2 * p + h_
                        nc.tensor.matmul(
                            out_ps[:],
                            vnat[:, sb, :],
                            pexp2[:, h_, :],
                            start=(sb == 0),
                            stop=False,
                        )
                # diagonal blocks, causal-trimmed
                for j in range(4):
                    sb = 4 * i + j
                    lo = 128 * j
                    sc2 = ps_sc.tile([128, 2, CHUNK], F32, tag="sc", name=f"sd{i}_{j}")
                    nc.tensor.matmul(
                        sc2[:, 0, lo:CHUNK],
                        krope0[:, ts(sb, 128)],
                        qkrope[0:H, i * CHUNK + lo : (i + 1) * CHUNK],
                        start=True,
                        stop=True,
                    )
                    pexp2 = pexpp.tile(
                        [128, 2, CHUNK], BF, tag="pexp", name=f"pd{i}_{j}"
                    )
                    nc.scalar.activation(
                        out=pexp2[:, 0, lo:CHUNK],
                        in_=sc2[:, 0, lo:CHUNK],
                        func=Exp,
                        scale=0.125,
                    )
                    nc.vector.tensor_mul(
                        pexp2[:, 0, lo : lo + 128],
                        pexp2[:, 0, lo : lo + 128],
                        trimask_sb[:],
                    )
                    nc.tensor.matmul(
                        out_ps[:, lo:CHUNK],
                        vnat[:, sb, :],
                        pexp2[:, 0, lo:CHUNK],
                        start=(sb == 0),
                        stop=(sb == nsb - 1),
                    )

                out_tiles[i] = out_ps

            def phase_c_norm(i):
                out_ps = out_tiles[i]
                # ---- normalize via PE transpose: per-partition den scalar
                outS = work.tile([H + 1, CHUNK], F32, tag="outS", name=f"oS{i}")
                nc.vector.tensor_copy(out=outS[:], in_=out_ps[:])
                tr_ps = ps_out.tile([128, 4, H + 1], F32, tag="out", name=f"tr{i}")
                for j in range(4):
                    nc.tensor.transpose(
                        tr_ps[:, j, :], outS[:, ts(j, 128)], identf_sb[:]
                    )
                recip4 = work.tile([128, 4], F32, tag="recip", name=f"r{i}")
                nc.vector.reciprocal(out=recip4[:], in_=tr_ps[:, :, H])
                out_sb = work.tile([128, 4, H], F32, tag="outN", name=f"oN{i}")
                for j in range(4):
                    nc.vector.tensor_scalar_mul(
                        out=out_sb[:, j, :],
                        in0=tr_ps[:, j, 0:H],
                        scalar1=recip4[:, j : j + 1],
                    )
                nc.sync.dma_start(
                    out=out_d.ap()[ts(i, CHUNK), :].rearrange(
                        "(j p) h -> p j h", p=128
                    ),
                    in_=out_sb[:],
                )

            # interleaved emission: PE-dense A fills ACT-bound gaps of C;
            # norms emitted late so they don't convoy the next chunk
            phase_a(0)
            phase_a(1)
            phase_b(0)
            phase_c_accum(0)
            phase_a(2)
            phase_c_norm(0)
            phase_a(3)
            phase_b(1)
            phase_c_accum(1)
            phase_c_norm(1)
            phase_c_accum(2)
            phase_c_norm(2)
            phase_c_accum(3)
            phase_c_norm(3)

    nc.compile()
    return nc


_NC_CACHE = None


def _get_nc():
    global _NC_CACHE
    if _NC_CACHE is None:
        _NC_CACHE = _build_bass()
    return _NC_CACHE


def make_in_maps(x, Wq, Wk, Wv):
    """Host-side prep: shard over batch + precompute constants."""
    coscos, sinsin, r2, trimask, identb2, ident_f32 = _build_consts()
    wqk = np.concatenate([Wq, Wk], axis=1).astype(bf16)  # (C, 128)
    wv = Wv.astype(bf16)
    wqkp = np.ascontiguousarray(wqk.reshape(NCB, 128, 128).transpose(1, 0, 2))
    wvp = np.ascontiguousarray(wv.reshape(NCB, 128, H).transpose(1, 0, 2))
    in_maps = []
    for b in range(B):
        xT = x[b].T.astype(bf16)  # (C, T)
        xTp = np.ascontiguousarray(
            xT.reshape(NCB, 128, NCHUNK, CHUNK).transpose(1, 2, 0, 3)
        )
        in_maps.append(
            {
                "xTp": xTp,
                "wqkp": wqkp,
                "wvp": wvp,
                "r2": r2,
                "coscos": coscos,
                "sinsin": sinsin,
                "trimask": trimask,
                "identb2": identb2,
                "identf": ident_f32,
            }
        )
    return in_maps


def kernel(x, Wq, Wk, Wv):
    from concourse.bass_utils import run_bass_kernel_spmd

    x = np.asarray(x, dtype=np.float32)
    Wq = np.asarray(Wq, dtype=np.float32)
    Wk = np.asarray(Wk, dtype=np.float32)
    Wv = np.asarray(Wv, dtype=np.float32)

    nc = _get_nc()
    in_maps = make_in_maps(x, Wq, Wk, Wv)
    res = run_bass_kernel_spmd(nc, in_maps, core_ids=list(range(NCORES)))
    out = np.stack([r["out"] for r in res.results])  # (B, T, H)
    return np.ascontiguousarray(out.astype(np.float32))


# revision 13
# speedup vs baseline: 1.0684x; 1.0684x over previous
"""Single-head causal attention with RoPE on 8 Trainium2 NeuronCores.

Problem: x:(8,2048,1024), Wq/Wk/Wv:(1024,64) -> out:(8,2048,64)
  q = rope(x@Wq); k = rope(x@Wk); v = x@Wv
  out = softmax(causal(q k^T / sqrt(64))) @ v

Sharding: data-parallel over batch B=8, one batch element per core.

Per-core dataflow (transposed [feature, time] layout so the softmax
reduction lands on the PSUM partition dim via a ones-column matmul):
  A(i): qkT = [Wq|Wk]^T @ xT_chunk (PE, M=128); rot = R2 @ bf16(qkT) (PE)
        q'k' = qkT*coscos + rot*sinsin (DVE) -> qkrope bf16 (q rows 0:64,
        k rows 64:128); krope0 partition copy via SBUF-SBUF DMA
        v-proj for chunk PAIRS col-tiled: two chunks run CONCURRENTLY in
        PE column groups 0:64 / 64:128 (independent moving xbus streams)
  B(g): v natural via PE transpose -> vnat[128,16,65] (ones col 64)
  C(i): full s-blocks in PAIRS sharing one 2-bank PSUM tile; ONE exp
        ACTIVATE covers both (halves ACT per-call overhead); diagonal
        blocks single + causal-trimmed + trimask
        [num;den] psum += vnat_sb^T @ pexp (PE, M=65; ones col -> den)
        normalize: evict (DVE), PE-transpose, reciprocal + tensor_scalar
  DMA queues by need-time: xT stream on sync; consts on gpsimd (SWDGE,
  ordered by first use); krope0 dups on scalar; outputs on vector.
"""

import numpy as np
import ml_dtypes

B, T, C, H = 8, 2048, 1024, 64
NCORES = 8
CHUNK = 512
NCHUNK = T // CHUNK  # 4
NSB = T // 128       # 16 s-blocks
NCB = C // 128       # 8 c-blocks

bf16 = ml_dtypes.bfloat16


# ---------------------------------------------------------------- host consts
def _build_consts():
    half = H // 2
    inv_freq = (1.0 / (10000.0 ** (np.arange(half, dtype=np.float32) / half))).astype(
        np.float32
    )
    t = np.arange(T, dtype=np.float32)
    freqs = t[:, None] * inv_freq[None, :]  # (T, half) fp32
    cos = np.repeat(np.cos(freqs), 2, axis=-1)  # (T, H)
    sin = np.repeat(np.sin(freqs), 2, axis=-1)
    cosT = np.ascontiguousarray(cos.T)  # (H, T)
    sinT = np.ascontiguousarray(sin.T)

    coscos = np.concatenate([cosT, cosT], axis=0).astype(bf16)  # (128, T)
    sinsin = np.concatenate([sinT, sinT], axis=0).astype(bf16)

    # rot = R @ q with rot[2i] = -q[2i+1], rot[2i+1] = q[2i]
    Rm = np.zeros((H, H), np.float32)
    for i in range(half):
        Rm[2 * i, 2 * i + 1] = -1.0
        Rm[2 * i + 1, 2 * i] = 1.0
    r2 = np.zeros((128, 128), np.float32)
    r2[0:H, 0:H] = Rm.T
    r2[H:128, H:128] = Rm.T
    r2 = r2.astype(bf16)

    sl = np.arange(128)
    trimask = (sl[:, None] <= sl[None, :]).astype(bf16)  # (128, 128)

    identb2 = np.concatenate([np.eye(H), np.eye(H)], axis=0).astype(bf16)  # (128, 64)
    ident_f32 = np.eye(H + 1, dtype=np.float32)  # (65, 65) for out transposes

    return coscos, sinsin, r2, trimask, identb2, ident_f32


# ---------------------------------------------------------------- bass program
def _build_bass():
    import concourse.mybir as mybir
    import concourse.tile as tile
    from concourse import bacc
    from concourse.bass import ts

    BF = mybir.dt.bfloat16
    F32 = mybir.dt.float32
    Exp = mybir.ActivationFunctionType.Exp

    nc = bacc.Bacc(
        "TRN2",
        target_bir_lowering=False,
        debug=False,
        enable_asserts=False,
        num_devices=NCORES,
    )

    # xT prepacked on host to SBUF layout [128(p), chunk, cblk, 512]
    xT_d = nc.dram_tensor("xTp", [128, NCHUNK, NCB, CHUNK], BF, kind="ExternalInput")
    wqk_d = nc.dram_tensor("wqkp", [128, NCB, 128], BF, kind="ExternalInput")
    wv_d = nc.dram_tensor("wvp", [128, NCB, H], BF, kind="ExternalInput")
    r2_d = nc.dram_tensor("r2", [128, 128], BF, kind="ExternalInput")
    coscos_d = nc.dram_tensor("coscos", [128, T], BF, kind="ExternalInput")
    sinsin_d = nc.dram_tensor("sinsin", [128, T], BF, kind="ExternalInput")
    trimask_d = nc.dram_tensor("trimask", [128, 128], BF, kind="ExternalInput")
    identb_d = nc.dram_tensor("identb2", [128, H], BF, kind="ExternalInput")
    identf_d = nc.dram_tensor("identf", [H + 1, H + 1], F32, kind="ExternalInput")
    out_d = nc.dram_tensor("out", [T, H], F32, kind="ExternalOutput")

    with tile.TileContext(nc) as tc:
        with (
            tc.tile_pool(name="persist", bufs=1) as persist,
            tc.tile_pool(name="work", bufs=2) as work,
            tc.tile_pool(name="pexpp", bufs=4) as pexpp,
            tc.tile_pool(name="ps_scratch", bufs=2, space="PSUM") as ps_scratch,
            tc.tile_pool(name="ps_sc", bufs=2, space="PSUM") as ps_sc,
            tc.tile_pool(name="ps_out", bufs=2, space="PSUM") as ps_out,
        ):
            # ---- persistent SBUF tensors
            wqk_sb = persist.tile([128, NCB, 128], BF)
            wv_sb = persist.tile([128, NCB, H], BF)
            r2_sb = persist.tile([128, 128], BF)
            coscos_sb = persist.tile([128, T], BF)
            sinsin_sb = persist.tile([128, T], BF)
            trimask_sb = persist.tile([128, 128], BF)
            identb_sb = persist.tile([128, H], BF)
            identf_sb = persist.tile([H + 1, H + 1], F32)
            xT_sb = persist.tile([128, NCHUNK, NCB, CHUNK], BF)
            qkrope = persist.tile([128, T], BF)   # q' rows 0:64, k' rows 64:128
            krope0 = persist.tile([H, T], BF)     # k' copy at partitions 0:64
            vT_sb = persist.tile([128, NCHUNK // 2, CHUNK], BF)  # stacked pairs
            vnat = persist.tile([128, NSB, H + 1], BF)

            # ---- sync HWDGE queue: wqk (gates first matmul), then xT stream
            # (chunk 0 split in half so projections start ~1.3us earlier)
            nc.sync.dma_start(out=wqk_sb[:], in_=wqk_d.ap())
            nc.sync.dma_start(out=xT_sb[:, 0, 0:4], in_=xT_d.ap()[:, 0, 0:4])
            nc.sync.dma_start(out=xT_sb[:, 0, 4:8], in_=xT_d.ap()[:, 0, 4:8])
            for i in range(1, NCHUNK):
                nc.sync.dma_start(out=xT_sb[:, i], in_=xT_d.ap()[:, i])
            # ---- scalar HWDGE queue: rope consts + early weights
            nc.scalar.dma_start(out=coscos_sb[:], in_=coscos_d.ap())
            nc.scalar.dma_start(out=sinsin_sb[:], in_=sinsin_d.ap())
            nc.scalar.dma_start(out=r2_sb[:], in_=r2_d.ap())
            nc.scalar.dma_start(out=wv_sb[:], in_=wv_d.ap())
            # ---- gpsimd (SWDGE) queue: small late-need consts
            nc.gpsimd.dma_start(out=identb_sb[:], in_=identb_d.ap())
            nc.gpsimd.dma_start(out=trimask_sb[:], in_=trimask_d.ap())
            nc.gpsimd.dma_start(out=identf_sb[:], in_=identf_d.ap())

            nc.vector.memset(vnat[:], 1.0)  # ones col (64); cols 0:64 overwritten

            # PE warmup: ~3.5us of junk matmuls on a zeroed tile so the HAM
            # clock-gate opens to 2.4GHz before the first real projection
            zwarm = persist.tile([128, CHUNK], BF)
            nc.vector.memset(zwarm[:], 0.0)
            warm_ps = ps_sc.tile([128, 2, CHUNK], F32, tag="sc", name="warm")
            for w in range(16):
                nc.tensor.matmul(
                    warm_ps[:, 0, :],
                    zwarm[:, 0:128],
                    zwarm[:],
                    start=(w == 0),
                    stop=(w == 15),
                )

            def phase_a(i):
                tsl = ts(i, CHUNK)
                qk_ps = ps_scratch.tile([128, CHUNK], F32, tag="scr", name=f"qk{i}")
                for c in range(NCB):
                    nc.tensor.matmul(
                        qk_ps[:],
                        wqk_sb[:, c, :],
                        xT_sb[:, i, c, :],
                        start=(c == 0),
                        stop=(c == NCB - 1),
                    )
                qkS = work.tile([128, CHUNK], BF, tag="qkS", name=f"qkS{i}")
                nc.vector.tensor_copy(out=qkS[:], in_=qk_ps[:])
                rot_ps = ps_scratch.tile([128, CHUNK], F32, tag="scr", name=f"rot{i}")
                nc.tensor.matmul(rot_ps[:], r2_sb[:], qkS[:], start=True, stop=True)

                tmp1 = work.tile([128, CHUNK], BF, tag="tmp1", name=f"t1_{i}")
                nc.vector.tensor_mul(tmp1[:], qkS[:], coscos_sb[:, tsl])
                tmp2 = work.tile([128, CHUNK], BF, tag="tmp2", name=f"t2_{i}")
                nc.vector.tensor_mul(tmp2[:], rot_ps[:], sinsin_sb[:, tsl])
                nc.vector.tensor_add(qkrope[:, tsl], tmp1[:], tmp2[:])
                # k' partition copy for use as scores lhsT (gpsimd queue --
                # keeps the ACT instruction stream free for pure exp work)
                nc.gpsimd.dma_start(out=krope0[:, tsl], in_=qkrope[H:128, tsl])

                if i % 2 == 1:
                    # v-projection for chunk pair (i-1, i), col-tiled pairs
                    # interleaved so both column groups stream concurrently
                    g = i // 2
                    v_ps = ps_scratch.tile([128, CHUNK], F32, tag="scr", name=f"v{g}")
                    for c in range(NCB):
                        nc.tensor.matmul(
                            v_ps[0:H, :],
                            wv_sb[:, c, :],
                            xT_sb[:, i - 1, c, :],
                            start=(c == 0),
                            stop=(c == NCB - 1),
                            skip_group_check=True,
                        )
                        nc.tensor.matmul(
                            v_ps[H:128, :],
                            wv_sb[:, c, :],
                            xT_sb[:, i, c, :],
                            start=(c == 0),
                            stop=(c == NCB - 1),
                            skip_group_check=True,
                        )
                    nc.vector.tensor_copy(out=vT_sb[:, g, :], in_=v_ps[:])

            def phase_b(g):
                # transpose 8 s-blocks of the vT pair group g (chunks 2g, 2g+1)
                for half_ in range(2):
                    vn_ps = ps_out.tile(
                        [128, 4, H], BF, tag="out", name=f"vn{g}_{half_}"
                    )
                    base = H * half_
                    for j in range(4):
                        nc.tensor.transpose(
                            vn_ps[:, j, :],
                            vT_sb[base : base + H, g, ts(j, 128)],
                            identb_sb[base : base + H, :],
                        )
                    first = 8 * g + 4 * half_
                    nc.vector.tensor_copy(
                        out=vnat[:, first : first + 4, 0:H], in_=vn_ps[:]
                    )

            out_tiles = {}

            def phase_c_accum(i):
                nsb = 4 * i + 4  # s-blocks 0 .. 4i+3 (0..4i-1 full, then diag)
                out_ps = ps_out.tile([H + 1, CHUNK], F32, tag="out", name=f"o{i}")
                out_tiles[i] = out_ps

                # units: full pairs then diagonal singles, software-pipelined
                # so scores(u+1) runs on PE during exp(u) on ACT
                units = [("pair", p) for p in range(2 * i)] + [
                    ("diag", j) for j in range(4)
                ]
                staged = []  # (kind, idx, sc_tile, pexp_tile)

                def emit_scores(u):
                    kind, idx = u
                    if kind == "pair":
                        sc2 = ps_sc.tile(
                            [128, 2, CHUNK], F32, tag="sc", name=f"s{i}_{idx}"
                        )
                        for h_ in range(2):
                            sb = 2 * idx + h_
                            nc.tensor.matmul(
                                sc2[:, h_, :],
                                krope0[:, ts(sb, 128)],
                                qkrope[0:H, ts(i, CHUNK)],
                                start=True,
                                stop=True,
                            )
                    else:
                        j = idx
                        lo = 128 * j
                        sc2 = ps_sc.tile(
                            [128, 2, CHUNK], F32, tag="sc", name=f"sd{i}_{j}"
                        )
                        nc.tensor.matmul(
                            sc2[:, 0, lo:CHUNK],
                            krope0[:, ts(4 * i + j, 128)],
                            qkrope[0:H, i * CHUNK + lo : (i + 1) * CHUNK],
                            start=True,
                            stop=True,
                        )
                    staged.append((kind, idx, sc2))

                def emit_exp_num(stage):
                    kind, idx, sc2 = stage
                    if kind == "pair":
                        pexp2 = pexpp.tile(
                            [128, 2, CHUNK], BF, tag="pexp", name=f"p{i}_{idx}"
                        )
                        nc.scalar.activation(
                            out=pexp2[:], in_=sc2[:], func=Exp, scale=0.125
                        )
                        for h_ in range(2):
                            sb = 2 * idx + h_
                            nc.tensor.matmul(
                                out_ps[:],
                                vnat[:, sb, :],
                                pexp2[:, h_, :],
                                start=(sb == 0),
                                stop=False,
                            )
                    else:
                        j = idx
                        sb = 4 * i + j
                        lo = 128 * j
                        pexp2 = pexpp.tile(
                            [128, 2, CHUNK], BF, tag="pexp", name=f"pd{i}_{j}"
                        )
                        nc.scalar.activation(
                            out=pexp2[:, 0, lo:CHUNK],
                            in_=sc2[:, 0, lo:CHUNK],
                            func=Exp,
                            scale=0.125,
                        )
                        nc.vector.tensor_mul(
                            pexp2[:, 0, lo : lo + 128],
                            pexp2[:, 0, lo : lo + 128],
                            trimask_sb[:],
                        )
                        nc.tensor.matmul(
                            out_ps[:, lo:CHUNK],
                            vnat[:, sb, :],
                            pexp2[:, 0, lo:CHUNK],
                            start=(sb == 0),
                            stop=(sb == nsb - 1),
                        )

                emit_scores(units[0])
                for u in units[1:]:
                    emit_scores(u)
                    emit_exp_num(staged.pop(0))
                emit_exp_num(staged.pop(0))

            def phase_c_norm(i):
                out_ps = out_tiles[i]
                # ---- normalize via PE transpose: per-partition den scalar
                outS = work.tile([H + 1, CHUNK], F32, tag="outS", name=f"oS{i}")
                nc.vector.tensor_copy(out=outS[:], in_=out_ps[:])
                tr_ps = ps_out.tile([128, 4, H + 1], F32, tag="out", name=f"tr{i}")
                for j in range(4):
                    nc.tensor.transpose(
                        tr_ps[:, j, :], outS[:, ts(j, 128)], identf_sb[:]
                    )
                recip4 = work.tile([128, 4], F32, tag="recip", name=f"r{i}")
                nc.vector.reciprocal(out=recip4[:], in_=tr_ps[:, :, H])
                out_sb = work.tile([128, 4, H], F32, tag="outN", name=f"oN{i}")
                for j in range(4):
                    nc.vector.tensor_scalar_mul(
                        out=out_sb[:, j, :],
                        in0=tr_ps[:, j, 0:H],
                        scalar1=recip4[:, j : j + 1],
                    )
                nc.sync.dma_start(
                    out=out_d.ap()[ts(i, CHUNK), :].rearrange(
                        "(j p) h -> p j h", p=128
                    ),
                    in_=out_sb[:],
                )

            # interleaved emission: PE-dense A fills ACT-bound gaps of C;
            # norms emitted late so they don't convoy the next chunk
            phase_a(0)
            phase_a(1)
            phase_b(0)
            phase_c_accum(0)
            phase_a(2)
            phase_c_norm(0)
            phase_a(3)
            phase_b(1)
            phase_c_accum(1)
            phase_c_norm(1)
            phase_c_accum(2)
            phase_c_norm(2)
            phase_c_accum(3)
            phase_c_norm(3)

    nc.compile()
    return nc


_NC_CACHE = None


def _get_nc():
    global _NC_CACHE
    if _NC_CACHE is None:
        _NC_CACHE = _build_bass()
    return _NC_CACHE


def make_in_maps(x, Wq, Wk, Wv):
    """Host-side prep: shard over batch + precompute constants."""
    coscos, sinsin, r2, trimask, identb2, ident_f32 = _build_consts()
    wqk = np.concatenate([Wq, Wk], axis=1).astype(bf16)  # (C, 128)
    wv = Wv.astype(bf16)
    wqkp = np.ascontiguousarray(wqk.reshape(NCB, 128, 128).transpose(1, 0, 2))
    wvp = np.ascontiguousarray(wv.reshape(NCB, 128, H).transpose(1, 0, 2))
    in_maps = []
    for b in range(B):
        xT = x[b].T.astype(bf16)  # (C, T)
        xTp = np.ascontiguousarray(
            xT.reshape(NCB, 128, NCHUNK, CHUNK).transpose(1, 2, 0, 3)
        )
        in_maps.append(
            {
                "xTp": xTp,
                "wqkp": wqkp,
                "wvp": wvp,
                "r2": r2,
                "coscos": coscos,
                "sinsin": sinsin,
                "trimask": trimask,
                "identb2": identb2,
                "identf": ident_f32,
            }
        )
    return in_maps


def kernel(x, Wq, Wk, Wv):
    from concourse.bass_utils import run_bass_kernel_spmd

    x = np.asarray(x, dtype=np.float32)
    Wq = np.asarray(Wq, dtype=np.float32)
    Wk = np.asarray(Wk, dtype=np.float32)
    Wv = np.asarray(Wv, dtype=np.float32)

    nc = _get_nc()
    in_maps = make_in_maps(x, Wq, Wk, Wv)
    res = run_bass_kernel_spmd(nc, in_maps, core_ids=list(range(NCORES)))
    out = np.stack([r["out"] for r in res.results])  # (B, T, H)
    return np.ascontiguousarray(out.astype(np.float32))
